# revision 21
# baseline (speedup 1.0000x reference)
"""Trainium2 Bass kernel for nn_KeySelect (sparse_attention), bf16 edition.

Sharding: 8 shards = (4 batches) x (2 spatial H-halves). Each core gets a
48-row padded slab (40 compute rows + 4-row halo each side, zero-filled
outside the image) and computes the full conv chain for its half with no
collectives; halo rows that would be wrong at the interior cut are computed
but discarded on the host (validity windows shrink by 1 per 3x3 conv and 4
for the 9x9 local-attention weighting).

Layouts: conv stages use [C<=128 partitions, 48*66] bf16 rows of 66
(1+64+1 cols; a 3x3 conv only needs +-1 col pads), so each 3x3 conv is 9
PSUM-accumulated bf16 matmuls at free-dim offsets dh*66+dw (1 cycle/row on
PE vs 4 for fp32). The weighting path uses a 48*72 layout (9x9 window
needs +-4 col pads); the conv2 eviction and the fold eviction convert
between the two with strided ACT writes (free).

Weighting out[c,p] = sum_k A[p,k] * x[c, p+d(k)] runs on DVE in bf16
(SBUF-only operands -> 2x mode): attention rows are replicated across
partitions by DMA partition-broadcast (no PE involvement), pairing taps k
and k+45 (constant offset 360 = 5*72) so all 128 partitions are used: upper
64 partitions accumulate tap k, lower 64 tap k+45 against a row-shifted
duplicate copy of x. Pair halves collapse via a tiny PE fold matmul.

nrep>1 builds the same program with the body repeated back-to-back
(test.py uses nrep=1 vs nrep=10 to measure per-execution HW time as a
slope, cancelling the multi-ms host/axon dispatch floor).

Output per core: 8 row-segment spatial sums of relu(bn5(conv5)) per co-half
[2,128,8] fp32 (segments partition local rows 4..43 so that segs 0..5 =
rows 4..35 and segs 2..7 = rows 12..43); the host picks the 32-valid-row
window per half, finishes the mean and the two tiny FCs.
"""

import numpy as np
import ml_dtypes

import concourse.bacc as bacc
import concourse.bass as bass
import concourse.mybir as mybir
from concourse import tile
from concourse.alu_op_type import AluOpType
from concourse.bass_utils import run_bass_kernel_spmd

F32 = mybir.dt.float32
BF16 = mybir.dt.bfloat16
NPBF = ml_dtypes.bfloat16
PADW, PADH = 72, 48
PADN = PADW * PADH  # 3456: weighting layout (9x9 window needs +-4 col pads)
CW = 66             # conv layout: 1+64+1 cols (3x3 window needs +-1 only)
CPADN = CW * PADH   # 3168
DUPB = 360          # lower-half row shift: delta(k+45)-delta(k) = 5*72
DUPN = 4096         # x2dup free size (slack for shifted reads)
ACC0 = 4 * PADW     # 288: first weighting element (row 4, col 0; 72-layout)
ACCN = 40 * PADW    # 2880: 40 output rows (72-layout)
ACC0C = 4 * CW      # subtract span (66-layout)
ACCNC = 40 * CW
BLOCKS = [(4, 7), (11, 7), (18, 7), (25, 7), (32, 7), (39, 5)]
# conv5 output row segments (block idx, row offset in block, nrows): the host
# only needs sum(rows 4..35) [top half] and sum(rows 12..43) [bottom half], so
# evict 8 segment sums per co-half instead of 40 per-row sums.
SEG5 = [(0, 0, 7), (1, 0, 1), (1, 1, 6), (2, 0, 7), (3, 0, 7), (4, 0, 4),
        (4, 4, 3), (5, 0, 5)]
SHIFTS = [(dh, dw) for dh in (-1, 0, 1) for dw in (-1, 0, 1)]
NPAIR = 45          # weighting groups: (k, k+45) for k<36, singles 36..44
BN_EPS = 1e-5
H = W = 64
ROWS = 40           # compute rows per core
HALO = 4


def _delta(k):
    return (k // 9 - 4) * PADW + (k % 9 - 4)


# ---------------------------------------------------------------- program --

def _build_program(nrep=1):
    nc = bacc.Bacc("TRN2", target_bir_lowering=False, debug=False)

    lk = nc.dram_tensor("lk", [8, 128, CPADN], BF16, kind="ExternalInput")[:]
    ln = nc.dram_tensor("ln", [8, 128, CPADN], BF16, kind="ExternalInput")[:]
    asb_d = nc.dram_tensor("asb", [NPAIR, 2, PADN], BF16, kind="ExternalInput")[:]
    w1 = nc.dram_tensor("w1", [8, 128, 2304], BF16, kind="ExternalInput")[:]
    w2 = nc.dram_tensor("w2", [128, 1152], BF16, kind="ExternalInput")[:]
    w3 = nc.dram_tensor("w3", [128, 1536], BF16, kind="ExternalInput")[:]
    w4 = nc.dram_tensor("w4", [8, 128, 2304], BF16, kind="ExternalInput")[:]
    w5 = nc.dram_tensor("w5", [2, 128, 2304], BF16, kind="ExternalInput")[:]
    fold_d = nc.dram_tensor("fold", [128, 64], BF16, kind="ExternalInput")[:]
    bnp_d = nc.dram_tensor("bnp", [128, 18], F32, kind="ExternalInput")[:]
    osum = nc.dram_tensor("osum", [2, 128, 8], F32, kind="ExternalOutput")[:]

    # bnp columns: [c1sc0,c1sc1,c1sh0,c1sh1, c2sc,c2sh, c3sc0,c3sc1,c3sh0,
    #               c3sh1, c4sc0,c4sc1,c4sh0,c4sh1, c5sc0,c5sc1,c5sh0,c5sh1]
    C1SC, C1SH, C2SC, C2SH = 0, 2, 4, 5
    C3SC, C3SH, C4SC, C4SH, C5SC, C5SH = 6, 8, 10, 12, 14, 16

    with tile.TileContext(nc) as tc:
        with (
            tc.tile_pool(name="sb", bufs=1) as sb,
            tc.tile_pool(name="cps", bufs=8, space="PSUM") as cps,
            tc.tile_pool(name="agp", bufs=3) as agp,
            tc.tile_pool(name="ttp", bufs=2) as ttp,
            tc.tile_pool(name="scr", bufs=4) as scrp,
        ):
            w2sb = sb.tile([128, 1152], BF16, name="w2sb", tag="w2sb")
            w3sb = sb.tile([128, 1536], BF16, name="w3sb", tag="w3sb")
            fold = sb.tile([128, 64], BF16, name="fold", tag="fold")
            bnp = sb.tile([128, 18], F32, name="bnp", tag="bnp")
            # small constants on the ACT queue (loaded once, reused by reps)
            nc.scalar.dma_start(w2sb[:], w2)
            nc.scalar.dma_start(w3sb[:], w3)
            nc.scalar.dma_start(fold[:], fold_d)
            nc.scalar.dma_start(bnp[:], bnp_d)

            for rep in range(nrep):
                _emit_body(nc, sb, cps, agp, ttp, scrp,
                           lk, ln, asb_d, w1, w4, w5, osum,
                           w2sb, w3sb, fold, bnp, first_rep=(rep == 0))

    nc.compile()
    return nc


def _emit_body(nc, sb, cps, agp, ttp, scrp, lk, ln, asb_d, w1, w4, w5, osum,
               w2sb, w3sb, fold, bnp, first_rep=True):
    C1SC, C1SH, C2SC, C2SH = 0, 2, 4, 5
    C3SC, C3SH, C4SC, C4SH, C5SC, C5SH = 6, 8, 10, 12, 14, 16
    if True:
        if True:
            slabs = [sb.tile([128, CPADN], BF16, name=f"slab{i}", tag=f"slab{i}") for i in range(2)]
            wst = [sb.tile([128, 2304], BF16, name=f"wst{i}", tag=f"wst{i}") for i in range(2)]
            c1 = [sb.tile([128, CPADN], BF16, name=f"c1_{i}", tag=f"c1_{i}") for i in range(2)]
            ybuf = [sb.tile([128, CPADN], BF16, name=f"y{i}", tag=f"y{i}") for i in range(2)]
            x2dup = sb.tile([128, DUPN], BF16, name="x2dup", tag="x2dup")
            accd = sb.tile([128, PADN], BF16, name="accd", tag="accd")
            wout = sb.tile([128, CPADN], BF16, name="wout", tag="wout")
            sums = sb.tile([128, 16], F32, name="sums", tag="sums")

            # first conv1 ktile loads lead the SP queue so PE starts ASAP;
            # the slab goes in two halves so early row-blocks unblock first
            nc.sync.dma_start(slabs[0][:, 0 : 24 * CW], lk[0][:, 0 : 24 * CW])
            nc.sync.dma_start(wst[0][:], w1[0])
            nc.sync.dma_start(slabs[0][:, 24 * CW : CPADN],
                              lk[0][:, 24 * CW : CPADN])
            # pad-region zero-init on Pool
            for t in (c1[0], c1[1], ybuf[0], ybuf[1], wout):
                nc.gpsimd.memset(t[:], 0.0)
            nc.gpsimd.memset(x2dup[:], 0.0)
            nc.gpsimd.memset(accd[:], 0.0)

            def r3c(ap):  # [P, n*66] -> [P, n, 66]
                return ap.rearrange("p (r c) -> p r c", c=CW)

            def r3w(ap):  # [P, n*72] -> [P, n, 72]
                return ap.rearrange("p (r c) -> p r c", c=PADW)

            def conv_mms(rhs_of, lhsT_of, nkt, psts, first, last):
                """nkt ktiles x 9 shifts x 6 blocks accumulated into psts.

                Shift-outer order keeps the stationary operand constant for
                the 6 consecutive block matmuls, minimizing weight switches
                on the PE weight-load path."""
                for ti in range(nkt):
                    rhs = rhs_of(ti)
                    for si, (dh, dw) in enumerate(SHIFTS):
                        for bi, (r0, nr) in enumerate(BLOCKS):
                            o = (r0 + dh) * CW + dw
                            nc.tensor.matmul(
                                out=psts[bi][:, : nr * CW],
                                lhsT=lhsT_of(ti, si),
                                rhs=rhs[:, o : o + nr * CW],
                                start=(first and ti == 0 and si == 0),
                                stop=(last and ti == nkt - 1 and si == 8),
                            )
                        yield ti, si

            def evict(psts, dst, m, sc_col, sh_col):
                """PSUM (66-layout) -> dst (66-layout), fused BN+ReLU."""
                for bi, (r0, nr) in enumerate(BLOCKS):
                    nc.scalar.activation(
                        out=r3c(dst[:m, 0:CPADN])[:, r0 : r0 + nr, 1:65],
                        in_=r3c(psts[bi][:m, : nr * CW])[:, :, 1:65],
                        func=mybir.ActivationFunctionType.Relu,
                        scale=bnp[:m, sc_col : sc_col + 1],
                        bias=bnp[:m, sh_col : sh_col + 1],
                    )

            def evict_to72(psts, dst, m, sc_col, sh_col):
                """PSUM (66-layout) -> dst (72-layout region), BN+ReLU."""
                for bi, (r0, nr) in enumerate(BLOCKS):
                    nc.scalar.activation(
                        out=r3w(dst[:m, 0:PADN])[:, r0 : r0 + nr, 4:68],
                        in_=r3c(psts[bi][:m, : nr * CW])[:, :, 1:65],
                        func=mybir.ActivationFunctionType.Relu,
                        scale=bnp[:m, sc_col : sc_col + 1],
                        bias=bnp[:m, sh_col : sh_col + 1],
                    )

            # ---- conv1: 1024 -> 256, rhs = lk slabs (streamed, 2 passes) --
            for co in range(2):
                psts = [cps.tile([128, nr * CW], F32, name="cp", tag="cp") for _, nr in BLOCKS]
                rhs_cache = {}

                def rhs_of(ti, _co=co):
                    if ti not in rhs_cache:
                        s = slabs[ti % 2]
                        if not (_co == 0 and ti == 0):
                            nc.sync.dma_start(s[:], lk[ti])
                            nc.sync.dma_start(wst[ti % 2][:], w1[ti])
                        rhs_cache[ti] = s[:]
                    return rhs_cache[ti]

                for _ in conv_mms(
                    rhs_of,
                    lambda ti, si, _co=co: wst[ti % 2][:, si * 256 + _co * 128 :
                                                       si * 256 + _co * 128 + 128],
                    8, psts, True, True,
                ):
                    pass
                evict(psts, c1[co][:], 128, C1SC + co, C1SH + co)

            # ---- conv2: 256 -> 64, rhs = c1, out -> x2dup upper @DUPB -----
            psts = [cps.tile([64, nr * CW], F32, name="cp", tag="cp") for _, nr in BLOCKS]
            for _ in conv_mms(
                lambda ti: c1[ti][:],
                lambda ti, si: w2sb[:, ti * 576 + si * 64 : ti * 576 + si * 64 + 64],
                2, psts, True, True,
            ):
                pass
            evict_to72(psts, x2dup[:, DUPB : DUPB + PADN], 64, C2SC, C2SH)
            # lower half copy: x2dup[64:128, q] = x2[q] (DMA: cross-partition)
            nc.scalar.dma_start(
                x2dup[64:128, 0:PADN], x2dup[0:64, DUPB : DUPB + PADN]
            )

            # ---- conv4 (y branch) interleaved with weighting bcast/FMA ----
            gdone = [0]

            def emit_wgroup(n):
                for _ in range(n):
                    if gdone[0] >= NPAIR:
                        return
                    g = gdone[0]
                    gdone[0] += 1
                    ag = agp.tile([128, ACCN], BF16, name="agrep", tag="agrep")
                    for hh in range(2):
                        nc.gpsimd.dma_start(
                            ag[hh * 64 : hh * 64 + 64, :],
                            asb_d[g][hh : hh + 1, ACC0 : ACC0 + ACCN]
                            .partition_broadcast(64),
                        )
                    x = DUPB + _delta(g) + ACC0
                    tt = ttp.tile([128, ACCN], BF16, name="wt", tag="wt")
                    nc.vector.tensor_tensor(
                        out=tt[:], in0=x2dup[:, x : x + ACCN], in1=ag[:],
                        op=AluOpType.mult,
                    )
                    nc.vector.tensor_tensor(
                        out=accd[:, ACC0 : ACC0 + ACCN],
                        in0=accd[:, ACC0 : ACC0 + ACCN],
                        in1=tt[:], op=AluOpType.add,
                    )

            for co in range(2):
                psts = [cps.tile([128, nr * CW], F32, name="cp", tag="cp") for _, nr in BLOCKS]
                rhs_cache = {}

                def rhs_of(ti):
                    if ti not in rhs_cache:
                        s = slabs[ti % 2]
                        nc.sync.dma_start(s[:], ln[ti])
                        nc.sync.dma_start(wst[ti % 2][:], w4[ti])
                        rhs_cache[ti] = s[:]
                    return rhs_cache[ti]

                for _ti, _bi in conv_mms(
                    rhs_of,
                    lambda ti, si, _co=co: wst[ti % 2][:, si * 256 + _co * 128 :
                                                       si * 256 + _co * 128 + 128],
                    8, psts, True, True,
                ):
                    emit_wgroup(1)
                evict(psts, ybuf[co][:], 128, C4SC + co, C4SH + co)
            emit_wgroup(NPAIR)

            # collapse pair halves: wout = acc_upper + acc_lower (PE fold),
            # converting 72-layout accd to 66-layout wout (6 rows per chunk)
            for ci in range(8):
                r0 = 6 * ci
                pc = cps.tile([64, 6 * PADW], F32, name="cp", tag="cp")
                nc.tensor.matmul(
                    out=pc[:], lhsT=fold[:],
                    rhs=accd[:, r0 * PADW : (r0 + 6) * PADW],
                    start=True, stop=True,
                )
                nc.scalar.activation(
                    out=r3c(wout[0:64, 0:CPADN])[:, r0 : r0 + 6, 1:65],
                    in_=r3w(pc[:])[:, :, 4:68],
                    func=mybir.ActivationFunctionType.Copy,
                )
            # lower half = wout shifted left by 2 rows (132): lets conv3 pair
            # shifts (-1,dw) and (+1,dw) in one K=128 matmul
            nc.scalar.dma_start(
                wout[64:128, 0 : CPADN - 132], wout[0:64, 132:CPADN]
            )

            # ---- conv3: 64 -> 256, rhs = wout (+shifted lower copy) -------
            # slots 0-2: K=128 pairs (-1,dw)&(+1,dw); slots 3-5: K=64 (0,dw)
            for co in range(2):
                psts = [cps.tile([128, nr * CW], F32, name="cp", tag="cp") for _, nr in BLOCKS]
                for j in range(6):
                    kk = 128 if j < 3 else 64
                    dw = (j % 3) - 1
                    for bi, (r0, nr) in enumerate(BLOCKS):
                        o = ((r0 - 1) if j < 3 else r0) * CW + dw
                        nc.tensor.matmul(
                            out=psts[bi][:, : nr * CW],
                            lhsT=w3sb[0:kk, j * 256 + co * 128 :
                                      j * 256 + co * 128 + 128],
                            rhs=wout[0:kk, o : o + nr * CW],
                            start=(j == 0),
                            stop=(j == 5),
                        )
                evict(psts, c1[co][:], 128, C3SC + co, C3SH + co)

            # ---- d = x3 - y (in place, valid rows incl. pad cols) ---------
            for co in range(2):
                nc.vector.tensor_tensor(
                    out=c1[co][:, ACC0C : ACC0C + ACCNC],
                    in0=c1[co][:, ACC0C : ACC0C + ACCNC],
                    in1=ybuf[co][:, ACC0C : ACC0C + ACCNC],
                    op=AluOpType.subtract,
                )

            # ---- conv5: 256 -> 256 + BN+ReLU + per-row sums ---------------
            nc.sync.dma_start(wst[0][:], w5[0])
            nc.sync.dma_start(wst[1][:], w5[1])
            for co in range(2):
                psts = [cps.tile([128, nr * CW], F32, name="cp", tag="cp") for _, nr in BLOCKS]
                for _ in conv_mms(
                    lambda ti: c1[ti][:],
                    lambda ti, si, _co=co: wst[ti][:, si * 256 + _co * 128 :
                                                   si * 256 + _co * 128 + 128],
                    2, psts, True, True,
                ):
                    pass
                for si_, (bi, off, nseg) in enumerate(SEG5):
                    nr = BLOCKS[bi][1]
                    c5s = scrp.tile([128, 7 * 64], F32, name="c5s", tag="c5s")
                    nc.scalar.activation(
                        out=c5s[:, : nseg * 64].rearrange(
                            "p (r c) -> p r c", c=64),
                        in_=r3c(psts[bi][:, : nr * CW])[:, off : off + nseg, 1:65],
                        func=mybir.ActivationFunctionType.Relu,
                        scale=bnp[:, C5SC + co : C5SC + co + 1],
                        bias=bnp[:, C5SH + co : C5SH + co + 1],
                        accum_out=sums[:, co * 8 + si_ : co * 8 + si_ + 1],
                    )
            for co in range(2):
                nc.scalar.dma_start(osum[co], sums[:, co * 8 : co * 8 + 8])


# ------------------------------------------------------------- host side --

def _pad_slab(x_bchw, g0):
    """[1024, 64, 64] -> [8, 128, 48*66] bf16, rows g0-4..g0+44 zero-pad."""
    out = np.zeros((1024, PADH, CW), np.float32)
    lo, hi = max(0, g0 - HALO), min(H, g0 + ROWS + HALO)
    out[:, lo - (g0 - HALO) : hi - (g0 - HALO), 1:65] = x_bchw[:, lo:hi, :]
    return out.reshape(8, 128, CPADN).astype(NPBF)


def _fold_bn(bn):
    g, b, m, v = [np.asarray(x, np.float32) for x in bn]
    sc = g / np.sqrt(v + BN_EPS)
    return sc, b - m * sc


def _pack_w3(w3t):
    """[64, 9*256] si-major -> [128, 6*256] packed slots: j<3 pairs
    si_a=dw+1 (rows 0:64) with si_b=6+dw+1 (rows 64:128, reads the
    +132-shifted wout copy); j>=3 singles si=3+dw+1 on rows 0:64."""
    out = np.zeros((128, 6 * 256), np.float32)
    for j in range(6):
        dw1 = j % 3
        if j < 3:
            out[0:64, j * 256:(j + 1) * 256] = w3t[:, (dw1) * 256:(dw1 + 1) * 256]
            out[64:128, j * 256:(j + 1) * 256] = w3t[:, (6 + dw1) * 256:(7 + dw1) * 256]
        else:
            out[0:64, j * 256:(j + 1) * 256] = w3t[:, (3 + dw1) * 256:(4 + dw1) * 256]
    return out.astype(NPBF)


def _wt(w, nkt):
    """[co, ci, 3, 3] -> [nkt, 128, 9*co] lhsT layout (free = si*Co + co)."""
    co, ci = w.shape[:2]
    return np.ascontiguousarray(
        w.reshape(co, nkt, ci // nkt, 9).transpose(1, 2, 3, 0)
    ).reshape(nkt, ci // nkt, 9 * co).astype(NPBF)


def prep_core_inputs(inputs, core):
    b, half = core // 2, core % 2
    g0 = half * 24  # local row r = global g0 + r; valid out rows differ
    lk = _pad_slab(np.asarray(inputs["low_key"][b], np.float32), g0)
    ln = _pad_slab(np.asarray(inputs["low_nonkey"][b], np.float32), g0)

    att = np.asarray(inputs["local_atten"][b], np.float32)  # [64, 64, 81]
    asb = np.zeros((NPAIR, 2, PADN), np.float32)
    a_loc = np.zeros((81, PADH, PADW), np.float32)
    a_loc[:, 4 : 4 + ROWS, 4:68] = att[g0 : g0 + ROWS].transpose(2, 0, 1)
    a_loc = a_loc.reshape(81, PADN)
    for g in range(NPAIR):
        asb[g, 0] = a_loc[g]
        if g + NPAIR < 81:
            asb[g, 1] = a_loc[g + NPAIR]

    w2t = _wt(np.asarray(inputs["w2"]), 2)  # [2, 128, 576]
    w2p = np.concatenate([w2t[0], w2t[1]], axis=1)  # [128, 1152]

    fold = np.zeros((128, 64), np.float32)
    fold[np.arange(64), np.arange(64)] = 1.0
    fold[64 + np.arange(64), np.arange(64)] = 1.0

    bnp = np.zeros((128, 18), np.float32)
    for i, (name, cols) in enumerate(
        [("bn1", (0, 2)), ("bn2", (4, 5)), ("bn3", (6, 8)),
         ("bn4", (10, 12)), ("bn5", (14, 16))]
    ):
        sc, sh = _fold_bn(np.asarray(inputs[name]))
        nco = sc.shape[0]
        if nco == 256:
            bnp[:, cols[0]] = sc[:128]
            bnp[:, cols[0] + 1] = sc[128:]
            bnp[:, cols[1]] = sh[:128]
            bnp[:, cols[1] + 1] = sh[128:]
        else:
            bnp[:64, cols[0]] = sc
            bnp[:64, cols[1]] = sh

    return {
        "lk": lk, "ln": ln, "asb": asb.astype(NPBF),
        "w1": _wt(np.asarray(inputs["w1"]), 8),
        "w2": w2p,
        "w3": _pack_w3(_wt(np.asarray(inputs["w3"]), 1)[0]),  # [128, 1536]
        "w4": _wt(np.asarray(inputs["w4"]), 8),
        "w5": _wt(np.asarray(inputs["w5"]), 2),
        "fold": fold.astype(NPBF), "bnp": bnp,
    }


def postprocess(osums, inputs):
    """osums: list of 8 arrays [2, 128, 8] of row-segment sums -> [4, 1].
    Segments cover local rows 4..43; top half uses segs 0..5 (rows 4..35 =
    global rows 0..31), bottom half segs 2..7 (rows 12..43 = 32..63)."""
    mean = np.zeros((4, 256), np.float32)
    for core in range(8):
        b, half = core // 2, core % 2
        s = np.asarray(osums[core], np.float32).reshape(256, 8)
        segs = slice(0, 6) if half == 0 else slice(2, 8)
        mean[b] += s[:, segs].sum(axis=1)
    mean /= float(H * W)
    fw1 = np.asarray(inputs["fw1"], np.float32)
    fb1 = np.asarray(inputs["fb1"], np.float32)
    fw2 = np.asarray(inputs["fw2"], np.float32)
    fb2 = np.asarray(inputs["fb2"], np.float32)
    out = mean @ fw1.T + fb1
    out = out @ fw2.T + fb2
    return out.astype(np.float32)


_prog_cache = {}
LAST = {}


def kernel(**inputs) -> np.ndarray:
    import os, time
    if "nc" not in _prog_cache:
        _prog_cache["nc"] = _build_program()
    nc = _prog_cache["nc"]
    in_maps = [prep_core_inputs(inputs, core) for core in range(8)]
    t0 = time.time()
    res = run_bass_kernel_spmd(
        nc, in_maps, list(range(8)), trace=bool(os.environ.get("KS_TRACE"))
    )
    LAST["spmd_s"] = time.time() - t0
    LAST["res"] = res
    return postprocess([r["osum"] for r in res.results], inputs)
